# revision 1
# baseline (speedup 1.0000x reference)
"""HQQ 4-bit quantized linear on 8 Trainium2 NeuronCores (Bass/Tile).

out[4096, 11008] = x[4096, 4096] @ dequant(W_q, scale, zero).T + bias

Key index fact: reference reshapes ((W_r - zero) * scale) from [64, 704512]
to [11008, 4096].  With o = output feature, i = input feature:
    o = g_row * 172 + j,   group g = j * 4096 + i,   g_row in [0, 64)
so sharding 8 consecutive g_rows per core gives each core a contiguous
1376-column output slice (column-parallel linear, x replicated).

Per-core pipeline:
  phase 1: DMA W_q rows + scale/zero, extract nibble, dequantize to fp16,
           PE-transpose into a resident [4096(i) x 1376(o)] fp16 W.T
  phase 2: stream x row-blocks, convert fp16, PE-transpose to x.T tiles,
           accumulate out[t-tile, o-tile] = sum_k x.T[k,t].T @ W.T[k,o]
           in PSUM (bias pre-loaded via a K=1 ones x bias matmul).
"""

import numpy as np
from contextlib import ExitStack

import concourse.bacc as bacc
import concourse.bass as bass
import concourse.mybir as mybir
import concourse.tile as tile
from concourse.bass_utils import run_bass_kernel_spmd

dt = mybir.dt
Alu = mybir.AluOpType

TOKENS, IN_F, OUT_F, GS = 4096, 4096, 11008, 64
G = OUT_F * IN_F // GS            # 704512 quantization groups
J = G // IN_F                     # 172 groups per (g_row, i) plane
NCORES = 8
RPC = GS // NCORES                # 8 g_rows per core
O_C = RPC * J                     # 1376 output cols per core
NT = TOKENS // 128                # 32 token tiles
NK = IN_F // 128                  # 32 contraction blocks
O_SPLITS = ((0, 512), (512, 512), (1024, 352))   # psum o-tiles (1 bank each)
NB = len(O_SPLITS)
IC = 512                          # i-chunk for dequant / x streaming
JSPLIT = ((0, 128), (128, J - 128))   # j=172 -> partitions 128 + 44

_CACHE = {}


def _build():
    nc = bacc.Bacc("TRN2", target_bir_lowering=False, debug=False,
                   num_devices=NCORES)

    x_d = nc.dram_tensor("x", [TOKENS, IN_F], dt.float32, kind="ExternalInput")
    q_d = nc.dram_tensor("wq", [RPC, J, IN_F], dt.int32, kind="ExternalInput")
    s_d = nc.dram_tensor("scale", [J, IN_F], dt.float32, kind="ExternalInput")
    z_d = nc.dram_tensor("zero", [J, IN_F], dt.float32, kind="ExternalInput")
    b_d = nc.dram_tensor("bias", [1, O_C], dt.float32, kind="ExternalInput")
    hs_d = nc.dram_tensor("hsel", [128, 1], dt.float32, kind="ExternalInput")
    ls_d = nc.dram_tensor("lsel", [128, 1], dt.float32, kind="ExternalInput")
    id_d = nc.dram_tensor("ident", [128, 128], dt.bfloat16, kind="ExternalInput")
    o_d = nc.dram_tensor("out", [TOKENS, O_C], dt.float32, kind="ExternalOutput")

    with ExitStack() as ctx:
        tc = ctx.enter_context(tile.TileContext(nc))
        const = ctx.enter_context(tc.tile_pool(name="const", bufs=1))
        ph1 = ctx.enter_context(tc.tile_pool(name="ph1", bufs=2))
        ph2 = ctx.enter_context(tc.tile_pool(name="ph2", bufs=8))
        xtp = ctx.enter_context(tc.tile_pool(name="xtp", bufs=16))
        opool = ctx.enter_context(tc.tile_pool(name="opool", bufs=4))
        pacc = ctx.enter_context(
            tc.tile_pool(name="pacc", bufs=2, space=bass.MemorySpace.PSUM))
        ptr = ctx.enter_context(
            tc.tile_pool(name="ptr", bufs=2, space=bass.MemorySpace.PSUM))

        ident = const.tile([128, 128], dt.bfloat16)
        nc.sync.dma_start(ident[:], id_d[:])
        hsel = const.tile([128, 1], dt.float32)
        nc.sync.dma_start(hsel[:], hs_d[:])
        lsel = const.tile([128, 1], dt.float32)
        nc.sync.dma_start(lsel[:], ls_d[:])
        biasf = const.tile([1, O_C], dt.float32)
        nc.sync.dma_start(biasf[:], b_d[:])
        biash = const.tile([1, O_C], dt.bfloat16)
        nc.scalar.copy(biash[:], biasf[:])
        ones = const.tile([1, 128], dt.bfloat16)
        nc.vector.memset(ones[:], 1.0)

        # resident transposed dequantized weights: [i-partition, k-block, o]
        WT = const.tile([128, NK, O_C], dt.bfloat16)

        # ---- phase 1: dequant + transpose W ----
        for ic in range(IN_F // IC):          # 8 chunks of 512 along i
            sz = {}
            for (jb, jn) in JSPLIT:
                st = ph1.tile([jn, IC], dt.float32, tag=f"s{jb}")
                zt = ph1.tile([jn, IC], dt.float32, tag=f"z{jb}")
                nc.sync.dma_start(st[:], s_d[jb:jb + jn, ic * IC:(ic + 1) * IC])
                nc.sync.dma_start(zt[:], z_d[jb:jb + jn, ic * IC:(ic + 1) * IC])
                zs = ph1.tile([jn, IC], dt.float32, tag=f"zs{jb}")
                nc.vector.tensor_mul(zs[:], zt[:], st[:])
                sz[jb] = (st, zs)
            for r in range(RPC):
                for (jb, jn) in JSPLIT:
                    st, zs = sz[jb]
                    q = ph1.tile([jn, IC], dt.int32, tag=f"q{jb}")
                    nc.sync.dma_start(
                        q[:], q_d[r, jb:jb + jn, ic * IC:(ic + 1) * IC])
                    # nib = hsel*((q - (q&15))/16) + lsel*(q&15)
                    #     = (hsel/16)*q + (lsel - hsel/16)*(q&15)
                    # with per-core constants A=hsel/16, B=lsel-hsel/16:
                    # exact in fp32 (q < 256), selects hi or lo nibble.
                    lo_i = ph1.tile([jn, IC], dt.int32, tag=f"li{jb}")
                    nc.vector.tensor_single_scalar(
                        lo_i[:], q[:], 15, Alu.bitwise_and)
                    t1 = ph1.tile([jn, IC], dt.float32, tag=f"t{jb}")
                    nc.vector.tensor_scalar_mul(t1[:], q[:], hsel[0:jn, 0:1])
                    nf = ph1.tile([jn, IC], dt.float32, tag=f"f{jb}")
                    nc.vector.scalar_tensor_tensor(
                        nf[:], lo_i[:], lsel[0:jn, 0:1], t1[:],
                        Alu.mult, Alu.add)
                    wh = ph1.tile([jn, IC], dt.bfloat16, tag=f"w{jb}")
                    nc.vector.tensor_mul(nf[:], nf[:], st[:])
                    nc.vector.tensor_sub(wh[:], nf[:], zs[:])  # fp16 out
                    for sub in range(IC // 128):
                        k = ic * (IC // 128) + sub
                        pt = ptr.tile([128, jn], dt.bfloat16, tag="tr")
                        nc.tensor.transpose(
                            pt[:, 0:jn], wh[0:jn, sub * 128:(sub + 1) * 128],
                            ident[0:jn, 0:jn])
                        nc.scalar.copy(
                            WT[:, k, r * J + jb:r * J + jb + jn], pt[:, 0:jn])

        # ---- phase 2: stream x, transpose, matmul ----
        for t in range(NT):
            acc = []
            for p, (ob_, on_) in enumerate(O_SPLITS):
                a = pacc.tile([128, on_], dt.float32, tag=f"a{p}")
                nc.tensor.matmul(
                    a[:], ones[0:1, :], biash[0:1, ob_:ob_ + on_],
                    start=True, stop=False)
                acc.append(a)
            for icc in range(IN_F // IC):
                xr = ph2.tile([128, IC], dt.float32, tag="xr")
                nc.sync.dma_start(
                    xr[:], x_d[t * 128:(t + 1) * 128, icc * IC:(icc + 1) * IC])
                xh = ph2.tile([128, IC], dt.bfloat16, tag="xh")
                nc.scalar.copy(xh[:], xr[:])
                for sub in range(IC // 128):
                    k = icc * (IC // 128) + sub
                    pt = ptr.tile([128, 128], dt.bfloat16, tag="tr")
                    nc.tensor.transpose(
                        pt[:], xh[:, sub * 128:(sub + 1) * 128], ident[:])
                    xT = xtp.tile([128, 128], dt.bfloat16, tag="xT")
                    nc.vector.tensor_copy(xT[:], pt[:])
                    for p, (ob_, on_) in enumerate(O_SPLITS):
                        nc.tensor.matmul(
                            acc[p][:], xT[:], WT[:, k, ob_:ob_ + on_],
                            start=False, stop=(k == NK - 1))
            for p, (ob_, on_) in enumerate(O_SPLITS):
                ob = opool.tile([128, on_], dt.float32, tag=f"o{p}")
                nc.vector.tensor_copy(ob[:], acc[p][:])
                nc.sync.dma_start(
                    o_d[t * 128:(t + 1) * 128, ob_:ob_ + on_], ob[:])

    nc.compile()
    return nc


def get_nc():
    if "nc" not in _CACHE:
        _CACHE["nc"] = _build()
    return _CACHE["nc"]


def make_in_maps(x, W_q, scale, zero, bias):
    x = np.ascontiguousarray(x, dtype=np.float32)
    W_q = np.ascontiguousarray(W_q, dtype=np.int32)
    s2 = np.ascontiguousarray(scale, dtype=np.float32).reshape(J, IN_F)
    z2 = np.ascontiguousarray(zero, dtype=np.float32).reshape(J, IN_F)
    import ml_dtypes
    bias = np.ascontiguousarray(bias, dtype=np.float32)
    ident = np.eye(128, dtype=ml_dtypes.bfloat16)
    in_maps = []
    for c in range(NCORES):
        r0 = RPC * (c % 4)
        in_maps.append({
            "x": x,
            "wq": np.ascontiguousarray(W_q[r0:r0 + RPC]).reshape(RPC, J, IN_F),
            "scale": s2,
            "zero": z2,
            "bias": bias[c * O_C:(c + 1) * O_C].reshape(1, O_C),
            "hsel": np.full((128, 1), 1.0 / 16 if c < 4 else 0.0,
                            dtype=np.float32),
            "lsel": np.full((128, 1), -1.0 / 16 if c < 4 else 1.0,
                            dtype=np.float32),
            "ident": ident,
        })
    return in_maps


def kernel(x, W_q, scale, zero, bias):
    nc = get_nc()
    in_maps = make_in_maps(x, W_q, scale, zero, bias)
    res = run_bass_kernel_spmd(nc, in_maps, list(range(NCORES)))
    return np.concatenate(
        [res.results[c]["out"] for c in range(NCORES)], axis=1)



# revision 2
# speedup vs baseline: 1.8996x; 1.8996x over previous
"""HQQ 4-bit quantized linear on 8 Trainium2 NeuronCores (Bass/Tile).

out[4096, 11008] = x[4096, 4096] @ dequant(W_q, scale, zero).T + bias

Column-parallel: core c owns g_rows [8c, 8c+8) of the 64-row nibble
matrix, i.e. the contiguous output slice o in [1376c, 1376c+1376).
Within a core, output col = r*172 + j (r = local g_row, j in [0,172)),
input i = k*128 + ii; group g = j*4096 + i.

Host staging (layout only - all arithmetic stays on device):
  - nibble of interest extracted to uint8 and pre-transposed to
    [k, ii, r, j] so dequant runs directly in the matmul layout
  - x pre-transposed/tiled to [tt, ii, k, tj] fp16
  - scale/zero transposed to [k, ii, j] fp16

Device per core:
  phase 1: per k-tile, W[ii, r, j] = (nib - zero_bcast) * scale_bcast
           on DVE (2 passes, broadcast APs along r) into resident fp16
           WT[128, 32, 8, 172].
  phase 2: per t-tile, one 1MB DMA of x.T tiles, then 32 k x 3 o-bank
           back-to-back fp16 matmuls accumulating in PSUM; bias added
           during the PSUM->SBUF copy on DVE (bias replicated across
           partitions once by a broadcast DMA).
"""

import numpy as np
from contextlib import ExitStack

import concourse.bacc as bacc
import concourse.bass as bass
import concourse.mybir as mybir
import concourse.tile as tile
from concourse.bass_utils import run_bass_kernel_spmd

dt = mybir.dt

TOKENS, IN_F, OUT_F, GS = 4096, 4096, 11008, 64
J = 172                               # groups per (g_row, i) plane
NCORES = 8
RPC = GS // NCORES                    # 8 g_rows per core
O_C = RPC * J                         # 1376 output cols per core
NT = TOKENS // 128                    # 32 token tiles
NK = IN_F // 128                      # 32 contraction tiles
O_SPLITS = ((0, 512), (512, 512), (1024, 352))   # psum o-tiles (1 bank each)

_CACHE = {}


def _build():
    nc = bacc.Bacc("TRN2", target_bir_lowering=False, debug=False,
                   num_devices=NCORES)

    nib_d = nc.dram_tensor("nib", [NK, 128, RPC, J], dt.uint8,
                           kind="ExternalInput")
    sc_d = nc.dram_tensor("sc", [NK, 128, J], dt.float16,
                          kind="ExternalInput")
    z_d = nc.dram_tensor("z", [NK, 128, J], dt.float16,
                         kind="ExternalInput")
    xt_d = nc.dram_tensor("xt", [NT, 128, NK, 128], dt.float16,
                          kind="ExternalInput")
    b_d = nc.dram_tensor("bias", [1, O_C], dt.float32, kind="ExternalInput")
    o_d = nc.dram_tensor("out", [TOKENS, O_C], dt.float32,
                         kind="ExternalOutput")

    with ExitStack() as ctx:
        tc = ctx.enter_context(tile.TileContext(nc))
        const = ctx.enter_context(tc.tile_pool(name="const", bufs=1))
        ph1 = ctx.enter_context(tc.tile_pool(name="ph1", bufs=3))
        xp = ctx.enter_context(tc.tile_pool(name="xp", bufs=3))
        op = ctx.enter_context(tc.tile_pool(name="op", bufs=3))
        pacc = ctx.enter_context(
            tc.tile_pool(name="pacc", bufs=2, space=bass.MemorySpace.PSUM))

        WT = const.tile([128, NK, RPC, J], dt.float16)   # resident W.T
        biasf = const.tile([128, O_C], dt.float32)
        nc.sync.dma_start(biasf[:], b_d[:].to_broadcast((128, O_C)))

        # ---- phase 1: dequantize into WT ----
        for k in range(NK):
            nib = ph1.tile([128, RPC, J], dt.uint8, tag="nib")
            nc.sync.dma_start(nib[:], nib_d[k])
            sct = ph1.tile([128, J], dt.float16, tag="sc")
            nc.sync.dma_start(sct[:], sc_d[k])
            zt = ph1.tile([128, J], dt.float16, tag="z")
            nc.sync.dma_start(zt[:], z_d[k])
            d = ph1.tile([128, RPC, J], dt.float32, tag="d")
            nc.vector.tensor_sub(
                d[:], nib[:], zt[:].unsqueeze(1).broadcast_to((128, RPC, J)))
            nc.vector.tensor_mul(
                WT[:, k], d[:],
                sct[:].unsqueeze(1).broadcast_to((128, RPC, J)))

        # ---- phase 2: stream x.T tiles, dense matmul ----
        for t in range(NT):
            xs = xp.tile([128, NK, 128], dt.float16, tag="xs")
            nc.sync.dma_start(xs[:], xt_d[t])
            acc = []
            for p, (ob, on) in enumerate(O_SPLITS):
                acc.append(pacc.tile([128, on], dt.float32,
                                     tag=f"a{p}", name=f"a{p}"))
            for k in range(NK):
                wk = WT[:, k].opt()            # flat [128, O_C] view
                for p, (ob, on) in enumerate(O_SPLITS):
                    nc.tensor.matmul(
                        acc[p][:], xs[:, k], wk[:, ob:ob + on],
                        start=(k == 0), stop=(k == NK - 1))
            ob_t = op.tile([128, O_C], dt.float32, tag="ob")
            for p, (ob, on) in enumerate(O_SPLITS):
                nc.vector.tensor_add(
                    ob_t[:, ob:ob + on], acc[p][:], biasf[:, ob:ob + on])
            nc.sync.dma_start(o_d[t * 128:(t + 1) * 128], ob_t[:])

    nc.compile()
    return nc


def get_nc():
    if "nc" not in _CACHE:
        _CACHE["nc"] = _build()
    return _CACHE["nc"]


def make_in_maps(x, W_q, scale, zero, bias):
    x = np.ascontiguousarray(x, dtype=np.float32)
    W_q = np.ascontiguousarray(W_q, dtype=np.int32)
    bias = np.ascontiguousarray(bias, dtype=np.float32)

    # x.T tiled: [tt, ii, k, tj] fp16
    xt = np.ascontiguousarray(
        x.T.reshape(NK, 128, NT, 128).transpose(2, 1, 0, 3)
    ).astype(np.float16)
    # scale/zero: [172, 4096] -> [k, ii, j] fp16
    s2 = scale.reshape(J, IN_F).T.reshape(NK, 128, J).astype(np.float16)
    z2 = zero.reshape(J, IN_F).T.reshape(NK, 128, J).astype(np.float16)
    s2 = np.ascontiguousarray(s2)
    z2 = np.ascontiguousarray(z2)

    in_maps = []
    for c in range(NCORES):
        r0 = RPC * (c % 4)
        q = W_q[r0:r0 + RPC]                       # [8, 704512]
        nib = ((q >> 4) if c < 4 else q) & 0xF     # hi rows for cores 0-3
        nib = nib.astype(np.uint8).reshape(RPC, J, IN_F)
        nib = np.ascontiguousarray(
            nib.transpose(2, 0, 1).reshape(NK, 128, RPC, J))
        in_maps.append({
            "nib": nib,
            "sc": s2,
            "z": z2,
            "xt": xt,
            "bias": bias[c * O_C:(c + 1) * O_C].reshape(1, O_C),
        })
    return in_maps


def kernel(x, W_q, scale, zero, bias):
    nc = get_nc()
    in_maps = make_in_maps(x, W_q, scale, zero, bias)
    res = run_bass_kernel_spmd(nc, in_maps, list(range(NCORES)))
    return np.concatenate(
        [res.results[c]["out"] for c in range(NCORES)], axis=1)


# revision 5
# speedup vs baseline: 1.9641x; 1.0339x over previous
"""HQQ 4-bit quantized linear on 8 Trainium2 NeuronCores (Bass/Tile).

out[4096, 11008] = x[4096, 4096] @ dequant(W_q, scale, zero).T + bias

Column-parallel: core c owns g_rows [8c, 8c+8) of the 64-row nibble
matrix, i.e. the contiguous output slice o in [1376c, 1376c+1376).
Within a core, output col = r*172 + j (r = local g_row, j in [0,172)),
input i = k*128 + ii; group g = j*4096 + i.

Host staging (layout only - all arithmetic stays on device):
  - nibble of interest extracted to uint8 and pre-transposed to
    [k, ii, r, j] so dequant runs directly in the matmul layout
  - x pre-transposed/tiled to [tt, ii, k, tj] fp16
  - scale/zero transposed to [k, ii, j] fp16

Device per core:
  phase 1: per k-tile, W[ii, r, j] = (nib - zero_bcast) * scale_bcast
           on DVE (2 passes, broadcast APs along r) into resident fp16
           WT[128, 32, 8, 172].
  phase 2: per t-tile, one 1MB DMA of x.T tiles, then 32 k x 3 o-bank
           back-to-back fp16 matmuls accumulating in PSUM; bias added
           during the PSUM->SBUF copy on DVE (bias replicated across
           partitions once by a broadcast DMA).
"""

import numpy as np
from contextlib import ExitStack

import concourse.bacc as bacc
import concourse.bass as bass
import concourse.mybir as mybir
import concourse.tile as tile
from concourse.bass_utils import run_bass_kernel_spmd

dt = mybir.dt

TOKENS, IN_F, OUT_F, GS = 4096, 4096, 11008, 64
J = 172                               # groups per (g_row, i) plane
NCORES = 8
RPC = GS // NCORES                    # 8 g_rows per core
O_C = RPC * J                         # 1376 output cols per core
NT = TOKENS // 128                    # 32 token tiles
NK = IN_F // 128                      # 32 contraction tiles
O_SPLITS = ((0, 512), (512, 512), (1024, 352))   # psum o-tiles (1 bank each)

_CACHE = {}


def _build():
    nc = bacc.Bacc("TRN2", target_bir_lowering=False, debug=False,
                   num_devices=NCORES)

    nib_d = nc.dram_tensor("nib", [NK, 128, RPC, J], dt.float16,
                           kind="ExternalInput")
    sc_d = nc.dram_tensor("sc", [NK, 128, J], dt.float16,
                          kind="ExternalInput")
    z_d = nc.dram_tensor("z", [NK, 128, J], dt.float16,
                         kind="ExternalInput")
    xt_d = nc.dram_tensor("xt", [NT, 128, NK, 128], dt.float16,
                          kind="ExternalInput")
    b_d = nc.dram_tensor("bias", [1, O_C], dt.float32, kind="ExternalInput")
    o_d = nc.dram_tensor("out", [TOKENS, O_C], dt.float32,
                         kind="ExternalOutput")

    with ExitStack() as ctx:
        tc = ctx.enter_context(tile.TileContext(nc))
        const = ctx.enter_context(tc.tile_pool(name="const", bufs=1))
        ph1 = ctx.enter_context(tc.tile_pool(name="ph1", bufs=3))
        xp = ctx.enter_context(tc.tile_pool(name="xp", bufs=3))
        op = ctx.enter_context(tc.tile_pool(name="op", bufs=3))
        pacc = ctx.enter_context(
            tc.tile_pool(name="pacc", bufs=2, space=bass.MemorySpace.PSUM))

        WT = const.tile([128, NK, RPC, J], dt.float16)   # resident W.T

        # prefetch the first x.T slabs ahead of the phase-1 DMA burst so
        # the PE can start t0/t1 as soon as the first k-tiles dequantize
        xs_pre = []
        for t in range(2):
            xs = xp.tile([128, NK, 128], dt.float16, tag="xs", name=f"xs{t}")
            nc.sync.dma_start(xs[:], xt_d[t])
            xs_pre.append(xs)

        # ---- phase 1: dequantize into WT (all-fp16 -> DVE 2x mode) ----
        for k in range(NK):
            nib = ph1.tile([128, RPC, J], dt.float16, tag="nib")
            nc.sync.dma_start(nib[:], nib_d[k])
            sct = ph1.tile([128, J], dt.float16, tag="sc")
            nc.sync.dma_start(sct[:], sc_d[k])
            zt = ph1.tile([128, J], dt.float16, tag="z")
            nc.sync.dma_start(zt[:], z_d[k])
            d = ph1.tile([128, RPC, J], dt.float16, tag="d")
            nc.vector.tensor_sub(
                d[:], nib[:], zt[:].unsqueeze(1).broadcast_to((128, RPC, J)))
            nc.vector.tensor_mul(
                WT[:, k], d[:],
                sct[:].unsqueeze(1).broadcast_to((128, RPC, J)))

        biasf = const.tile([128, O_C], dt.float32)
        nc.sync.dma_start(biasf[:], b_d[:].to_broadcast((128, O_C)))

        # ---- phase 2: stream x.T tiles, dense matmul ----
        for t in range(NT):
            if t < 2:
                xs = xs_pre[t]
            else:
                xs = xp.tile([128, NK, 128], dt.float16, tag="xs", name="xs")
                nc.sync.dma_start(xs[:], xt_d[t])
            acc = []
            for p, (ob, on) in enumerate(O_SPLITS):
                acc.append(pacc.tile([128, on], dt.float32,
                                     tag=f"a{p}", name=f"a{p}"))
            for k in range(NK):
                wk = WT[:, k].opt()            # flat [128, O_C] view
                for p, (ob, on) in enumerate(O_SPLITS):
                    nc.tensor.matmul(
                        acc[p][:], xs[:, k], wk[:, ob:ob + on],
                        start=(k == 0), stop=(k == NK - 1))
            for p, (ob, on) in enumerate(O_SPLITS):
                obp = op.tile([128, on], dt.float32, tag=f"ob{p}",
                              name=f"ob{p}")
                nc.vector.tensor_add(obp[:], acc[p][:], biasf[:, ob:ob + on])
                nc.sync.dma_start(
                    o_d[t * 128:(t + 1) * 128, ob:ob + on], obp[:])

    nc.compile()
    return nc


def get_nc():
    if "nc" not in _CACHE:
        _CACHE["nc"] = _build()
    return _CACHE["nc"]


def make_in_maps(x, W_q, scale, zero, bias):
    x = np.ascontiguousarray(x, dtype=np.float32)
    W_q = np.ascontiguousarray(W_q, dtype=np.int32)
    bias = np.ascontiguousarray(bias, dtype=np.float32)

    # x.T tiled: [tt, ii, k, tj] fp16
    xt = np.ascontiguousarray(
        x.T.reshape(NK, 128, NT, 128).transpose(2, 1, 0, 3)
    ).astype(np.float16)
    # scale/zero: [172, 4096] -> [k, ii, j] fp16
    s2 = scale.reshape(J, IN_F).T.reshape(NK, 128, J).astype(np.float16)
    z2 = zero.reshape(J, IN_F).T.reshape(NK, 128, J).astype(np.float16)
    s2 = np.ascontiguousarray(s2)
    z2 = np.ascontiguousarray(z2)

    in_maps = []
    for c in range(NCORES):
        r0 = RPC * (c % 4)
        q = W_q[r0:r0 + RPC]                       # [8, 704512]
        nib = ((q >> 4) if c < 4 else q) & 0xF     # hi rows for cores 0-3
        nib = nib.astype(np.float16).reshape(RPC, J, IN_F)
        nib = np.ascontiguousarray(
            nib.transpose(2, 0, 1).reshape(NK, 128, RPC, J))
        in_maps.append({
            "nib": nib,
            "sc": s2,
            "z": z2,
            "xt": xt,
            "bias": bias[c * O_C:(c + 1) * O_C].reshape(1, O_C),
        })
    return in_maps


def kernel(x, W_q, scale, zero, bias):
    nc = get_nc()
    in_maps = make_in_maps(x, W_q, scale, zero, bias)
    res = run_bass_kernel_spmd(nc, in_maps, list(range(NCORES)))
    return np.concatenate(
        [res.results[c]["out"] for c in range(NCORES)], axis=1)


# revision 6
# speedup vs baseline: 2.0093x; 1.0230x over previous
"""HQQ 4-bit quantized linear on 8 Trainium2 NeuronCores (Bass/Tile).

out[4096, 11008] = x[4096, 4096] @ dequant(W_q, scale, zero).T + bias

Column-parallel: core c owns g_rows [8c, 8c+8) of the 64-row nibble
matrix, i.e. the contiguous output slice o in [1376c, 1376c+1376).
Within a core, output col = r*172 + j (r = local g_row, j in [0,172)),
input i = k*128 + ii; group g = j*4096 + i.

Host staging (layout/bit-extract only - all arithmetic stays on device):
  - nibble of interest extracted to uint8 and pre-transposed to
    [k, ii, r, j] so dequant runs directly in the matmul layout
  - x pre-transposed/tiled to [tt, ii, k, tj] fp16
  - scale/zero transposed to [k, ii, j] fp16

Device per core:
  phase 1: per k-tile, ACT casts nib u8->fp16, then DVE computes
           W = (nib - zero_bcast) * scale_bcast in two all-fp16 passes
           (2x DVE mode, broadcast APs along r) into resident fp16
           WT[128, 32, 8, 172].  Interleaved with the dequant, the PE
           accumulates t-tiles 0,1 (all 3 o-banks) and t-tile 2's first
           two o-banks -- all 8 PSUM banks -- so the PE stays ~busy
           through phase 1.
  phase 2: per t-tile, one 1MB DMA of x.T tiles, then 32k x 3 o-bank
           back-to-back fp16 matmuls accumulating in PSUM; bias added
           during the PSUM->SBUF copy on DVE (bias replicated across
           partitions once by a broadcast DMA).
"""

import numpy as np
from contextlib import ExitStack

import concourse.bacc as bacc
import concourse.bass as bass
import concourse.mybir as mybir
import concourse.tile as tile
from concourse.bass_utils import run_bass_kernel_spmd

dt = mybir.dt

TOKENS, IN_F, OUT_F, GS = 4096, 4096, 11008, 64
J = 172                               # groups per (g_row, i) plane
NCORES = 8
RPC = GS // NCORES                    # 8 g_rows per core
O_C = RPC * J                         # 1376 output cols per core
NT = TOKENS // 128                    # 32 token tiles
NK = IN_F // 128                      # 32 contraction tiles
O_SPLITS = ((0, 512), (512, 512), (1024, 352))   # psum o-tiles (1 bank each)

_CACHE = {}


def _build():
    nc = bacc.Bacc("TRN2", target_bir_lowering=False, debug=False,
                   num_devices=NCORES)

    nib_d = nc.dram_tensor("nib", [NK, 128, RPC, J], dt.uint8,
                           kind="ExternalInput")
    sc_d = nc.dram_tensor("sc", [NK, 128, J], dt.float16,
                          kind="ExternalInput")
    z_d = nc.dram_tensor("z", [NK, 128, J], dt.float16,
                         kind="ExternalInput")
    xt_d = nc.dram_tensor("xt", [NT, 128, NK, 128], dt.float16,
                          kind="ExternalInput")
    b_d = nc.dram_tensor("bias", [1, O_C], dt.float32, kind="ExternalInput")
    o_d = nc.dram_tensor("out", [TOKENS, O_C], dt.float32,
                         kind="ExternalOutput")

    with ExitStack() as ctx:
        tc = ctx.enter_context(tile.TileContext(nc))
        const = ctx.enter_context(tc.tile_pool(name="const", bufs=1))
        ph1 = ctx.enter_context(tc.tile_pool(name="ph1", bufs=3))
        xp = ctx.enter_context(tc.tile_pool(name="xp", bufs=3))
        op = ctx.enter_context(tc.tile_pool(name="op", bufs=3))
        pacc = ctx.enter_context(
            tc.tile_pool(name="pacc", bufs=2, space=bass.MemorySpace.PSUM))
        pacc2 = ctx.enter_context(
            tc.tile_pool(name="pacc2", bufs=1, space=bass.MemorySpace.PSUM))

        WT = const.tile([128, NK, RPC, J], dt.float16)   # resident W.T

        def fetch_k(k):
            nib = ph1.tile([128, RPC, J], dt.uint8, tag="nib", name="nib")
            nc.sync.dma_start(nib[:], nib_d[k])
            sct = ph1.tile([128, J], dt.float16, tag="sc", name="sct")
            nc.sync.dma_start(sct[:], sc_d[k])
            zt = ph1.tile([128, J], dt.float16, tag="z", name="zt")
            nc.sync.dma_start(zt[:], z_d[k])
            return nib, sct, zt

        # first dequant tiles ahead of the big x.T slabs so the DVE can
        # start immediately
        pre = {k: fetch_k(k) for k in range(2)}

        xs_map = {}
        for t in range(3):
            xs = xp.tile([128, NK, 128], dt.float16, tag="xs", name=f"xs{t}")
            nc.sync.dma_start(xs[:], xt_d[t])
            xs_map[t] = xs

        biasf = const.tile([128, O_C], dt.float32)
        nc.sync.dma_start(biasf[:], b_d[:].to_broadcast((128, O_C)))

        accs = {}
        for t in (0, 1):
            accs[t] = [pacc.tile([128, on], dt.float32, tag=f"a{p}",
                                 name=f"a{p}")
                       for p, (ob, on) in enumerate(O_SPLITS)]
        x01 = pacc2.tile([128, 1024], dt.float32, name="x01")

        # ---- phase 1: dequantize into WT + early matmuls (8 psum banks)
        wk_flat = {}
        for k in range(NK):
            nib, sct, zt = pre.pop(k) if k in pre else fetch_k(k)
            nibf = ph1.tile([128, RPC, J], dt.float16, tag="nibf",
                            name="nibf")
            nc.scalar.copy(nibf[:], nib[:])
            d = ph1.tile([128, RPC, J], dt.float16, tag="d", name="d")
            nc.vector.tensor_sub(
                d[:], nibf[:],
                zt[:].unsqueeze(1).broadcast_to((128, RPC, J)))
            nc.vector.tensor_mul(
                WT[:, k], d[:],
                sct[:].unsqueeze(1).broadcast_to((128, RPC, J)))
            wk = WT[:, k].opt()            # flat [128, O_C] view
            wk_flat[k] = wk
            se = dict(start=(k == 0), stop=(k == NK - 1))
            for t in (0, 1):
                for p, (ob, on) in enumerate(O_SPLITS):
                    nc.tensor.matmul(accs[t][p][:], xs_map[t][:, k],
                                     wk[:, ob:ob + on], **se)
            nc.tensor.matmul(x01[:, 0:512], xs_map[2][:, k],
                             wk[:, 0:512], **se)
            nc.tensor.matmul(x01[:, 512:1024], xs_map[2][:, k],
                             wk[:, 512:1024], **se)

        def copy_out(t, psums):
            for p, (ap, ob, on) in enumerate(psums):
                obp = op.tile([128, on], dt.float32, tag=f"ob{p}",
                              name=f"ob{p}")
                nc.vector.tensor_add(obp[:], ap, biasf[:, ob:ob + on])
                nc.sync.dma_start(
                    o_d[t * 128:(t + 1) * 128, ob:ob + on], obp[:])

        for t in (0, 1):
            copy_out(t, [(accs[t][p][:], ob, on)
                         for p, (ob, on) in enumerate(O_SPLITS)])

        # t=2: finish its third o-bank, then copy out
        a2t2 = pacc.tile([128, 352], dt.float32, tag="a2", name="a2")
        for k in range(NK):
            nc.tensor.matmul(a2t2[:], xs_map[2][:, k],
                             wk_flat[k][:, 1024:1376],
                             start=(k == 0), stop=(k == NK - 1))
        copy_out(2, [(x01[:, 0:512], 0, 512),
                     (x01[:, 512:1024], 512, 512),
                     (a2t2[:], 1024, 352)])

        # ---- phase 2: remaining t-tiles, dense matmul stream ----
        for t in range(3, NT):
            xs = xp.tile([128, NK, 128], dt.float16, tag="xs", name="xs")
            nc.sync.dma_start(xs[:], xt_d[t])
            acc = [pacc.tile([128, on], dt.float32, tag=f"a{p}",
                             name=f"a{p}")
                   for p, (ob, on) in enumerate(O_SPLITS)]
            for k in range(NK):
                wk = wk_flat[k]
                for p, (ob, on) in enumerate(O_SPLITS):
                    nc.tensor.matmul(
                        acc[p][:], xs[:, k], wk[:, ob:ob + on],
                        start=(k == 0), stop=(k == NK - 1))
            copy_out(t, [(acc[p][:], ob, on)
                         for p, (ob, on) in enumerate(O_SPLITS)])

    nc.compile()
    return nc


def get_nc():
    if "nc" not in _CACHE:
        _CACHE["nc"] = _build()
    return _CACHE["nc"]


def make_in_maps(x, W_q, scale, zero, bias):
    x = np.ascontiguousarray(x, dtype=np.float32)
    W_q = np.ascontiguousarray(W_q, dtype=np.int32)
    bias = np.ascontiguousarray(bias, dtype=np.float32)

    # x.T tiled: [tt, ii, k, tj] fp16
    xt = np.ascontiguousarray(
        x.T.reshape(NK, 128, NT, 128).transpose(2, 1, 0, 3)
    ).astype(np.float16)
    # scale/zero: [172, 4096] -> [k, ii, j] fp16
    s2 = scale.reshape(J, IN_F).T.reshape(NK, 128, J).astype(np.float16)
    z2 = zero.reshape(J, IN_F).T.reshape(NK, 128, J).astype(np.float16)
    s2 = np.ascontiguousarray(s2)
    z2 = np.ascontiguousarray(z2)

    in_maps = []
    for c in range(NCORES):
        r0 = RPC * (c % 4)
        q = W_q[r0:r0 + RPC]                       # [8, 704512]
        nib = ((q >> 4) if c < 4 else q) & 0xF     # hi rows for cores 0-3
        nib = nib.astype(np.uint8).reshape(RPC, J, IN_F)
        nib = np.ascontiguousarray(
            nib.transpose(2, 0, 1).reshape(NK, 128, RPC, J))
        in_maps.append({
            "nib": nib,
            "sc": s2,
            "z": z2,
            "xt": xt,
            "bias": bias[c * O_C:(c + 1) * O_C].reshape(1, O_C),
        })
    return in_maps


def kernel(x, W_q, scale, zero, bias):
    nc = get_nc()
    in_maps = make_in_maps(x, W_q, scale, zero, bias)
    res = run_bass_kernel_spmd(nc, in_maps, list(range(NCORES)))
    return np.concatenate(
        [res.results[c]["out"] for c in range(NCORES)], axis=1)


# revision 15
# speedup vs baseline: 2.0233x; 1.0070x over previous
"""HQQ 4-bit quantized linear on 8 Trainium2 NeuronCores (Bass/Tile).

out[4096, 11008] = x[4096, 4096] @ dequant(W_q, scale, zero).T + bias

Column-parallel: core c owns g_rows [8c, 8c+8) of the 64-row nibble
matrix, i.e. the contiguous output slice o in [1376c, 1376c+1376).
Within a core, output col = r*172 + j (r = local g_row, j in [0,172)),
input i = k*128 + ii; group g = j*4096 + i.

Host staging (layout/bit-extract only - all arithmetic stays on device):
  - nibble of interest extracted to uint8 and pre-transposed to
    [k, ii, r, j] so dequant runs directly in the matmul layout
  - x pre-transposed/tiled to [tt, ii, k, tj] fp16
  - scale/zero transposed to [k, ii, j] fp16

Device per core:
  phase 1: per k-tile, ACT casts nib u8->fp16, then DVE computes
           W = (nib - zero_bcast) * scale_bcast in two all-fp16 passes
           (2x DVE mode, broadcast APs along r) into resident fp16
           WT[128, 32, 8, 172].  Interleaved with the dequant, the PE
           accumulates t-tiles 0,1 (all 3 o-banks) and t-tile 2's first
           two o-banks -- all 8 PSUM banks -- so the PE stays ~busy
           through phase 1.
  phase 2: per t-tile, one 1MB DMA of x.T tiles, then 32k x 3 o-bank
           back-to-back fp16 matmuls accumulating in PSUM; bias added
           during the PSUM->SBUF copy on DVE (bias replicated across
           partitions once by a broadcast DMA).
"""

import numpy as np
from contextlib import ExitStack

import concourse.bacc as bacc
import concourse.bass as bass
import concourse.mybir as mybir
import concourse.tile as tile
from concourse.bass_utils import run_bass_kernel_spmd

dt = mybir.dt

TOKENS, IN_F, OUT_F, GS = 4096, 4096, 11008, 64
J = 172                               # groups per (g_row, i) plane
NCORES = 8
RPC = GS // NCORES                    # 8 g_rows per core
O_C = RPC * J                         # 1376 output cols per core
NT = TOKENS // 128                    # 32 token tiles
NK = IN_F // 128                      # 32 contraction tiles
O_SPLITS = ((0, 512), (512, 512), (1024, 352))   # psum o-tiles (1 bank each)
KCH = 2                               # k-tiles dequantized per DVE pass

_CACHE = {}


def _build():
    nc = bacc.Bacc("TRN2", target_bir_lowering=False, debug=False,
                   num_devices=NCORES)

    NCH = NK // KCH
    nib_d = nc.dram_tensor("nib", [NCH, 128, KCH, RPC, J], dt.uint8,
                           kind="ExternalInput")
    sc_d = nc.dram_tensor("sc", [NCH, 128, KCH, J], dt.float16,
                          kind="ExternalInput")
    z_d = nc.dram_tensor("z", [NCH, 128, KCH, J], dt.float16,
                         kind="ExternalInput")
    xt_d = nc.dram_tensor("xt", [NT, 128, NK, 128], dt.float16,
                          kind="ExternalInput")
    b_d = nc.dram_tensor("bias", [1, O_C], dt.float32, kind="ExternalInput")
    o_d = nc.dram_tensor("out", [TOKENS, O_C], dt.float32,
                         kind="ExternalOutput")

    with ExitStack() as ctx:
        tc = ctx.enter_context(tile.TileContext(nc))
        const = ctx.enter_context(tc.tile_pool(name="const", bufs=1))
        ph1 = ctx.enter_context(tc.tile_pool(name="ph1", bufs=3))
        xp = ctx.enter_context(tc.tile_pool(name="xp", bufs=3))
        op = ctx.enter_context(tc.tile_pool(name="op", bufs=3))
        pacc = ctx.enter_context(
            tc.tile_pool(name="pacc", bufs=2, space=bass.MemorySpace.PSUM))
        pacc2 = ctx.enter_context(
            tc.tile_pool(name="pacc2", bufs=1, space=bass.MemorySpace.PSUM))

        WT = const.tile([128, NK, RPC, J], dt.float16)   # resident W.T

        def fetch_ch(c):
            nib = ph1.tile([128, KCH, RPC, J], dt.uint8, tag="nib",
                           name="nib")
            nc.sync.dma_start(nib[:], nib_d[c])
            sct = ph1.tile([128, KCH, J], dt.float16, tag="sc", name="sct")
            nc.sync.dma_start(sct[:], sc_d[c])
            zt = ph1.tile([128, KCH, J], dt.float16, tag="z", name="zt")
            nc.sync.dma_start(zt[:], z_d[c])
            return nib, sct, zt

        # first dequant chunks ahead of the big x.T slabs so the DVE can
        # start immediately
        pre = {c: fetch_ch(c) for c in range(2)}

        xs_map = {}
        for t in range(3):
            xs = xp.tile([128, NK, 128], dt.float16, tag="xs", name=f"xs{t}")
            nc.sync.dma_start(xs[:], xt_d[t])
            xs_map[t] = xs

        biasf = const.tile([128, O_C], dt.float32)
        nc.sync.dma_start(biasf[:], b_d[:].to_broadcast((128, O_C)))

        accs = {}
        for t in (0, 1):
            accs[t] = [pacc.tile([128, on], dt.float32, tag=f"a{p}",
                                 name=f"a{p}")
                       for p, (ob, on) in enumerate(O_SPLITS)]
        x01 = pacc2.tile([128, 1024], dt.float32, name="x01")

        # ---- phase 1: dequantize into WT + early matmuls (8 psum banks)
        wk_flat = {}
        NCH = NK // KCH
        for c in range(NCH):
            nib, sct, zt = pre.pop(c) if c in pre else fetch_ch(c)
            nibf = ph1.tile([128, KCH, RPC, J], dt.float16, tag="nibf",
                            name="nibf")
            nc.scalar.copy(nibf[:], nib[:])
            d = ph1.tile([128, KCH, RPC, J], dt.float16, tag="d", name="d")
            nc.vector.tensor_sub(
                d[:], nibf[:],
                zt[:].unsqueeze(2).broadcast_to((128, KCH, RPC, J)))
            nc.vector.tensor_mul(
                WT[:, c * KCH:(c + 1) * KCH], d[:],
                sct[:].unsqueeze(2).broadcast_to((128, KCH, RPC, J)))
            for k in range(c * KCH, (c + 1) * KCH):
                wk = WT[:, k].opt()        # flat [128, O_C] view
                wk_flat[k] = wk
                se = dict(start=(k == 0), stop=(k == NK - 1))
                for t in (0, 1):
                    for p, (ob, on) in enumerate(O_SPLITS):
                        nc.tensor.matmul(accs[t][p][:], xs_map[t][:, k],
                                         wk[:, ob:ob + on], **se)
                nc.tensor.matmul(x01[:, 0:512], xs_map[2][:, k],
                                 wk[:, 0:512], **se)
                nc.tensor.matmul(x01[:, 512:1024], xs_map[2][:, k],
                                 wk[:, 512:1024], **se)

        def copy_out(t, psums, chunked=False):
            for p, (ap, ob, on) in enumerate(psums):
                if not chunked:
                    obp = op.tile([128, on], dt.float32, tag=f"ob{p}",
                                  name=f"ob{p}")
                    nc.vector.tensor_add(obp[:], ap, biasf[:, ob:ob + on])
                    nc.sync.dma_start(
                        o_d[t * 128:(t + 1) * 128, ob:ob + on], obp[:])
                    continue
                h = on // 2
                for s, (cb, cn) in enumerate(((0, h), (h, on - h))):
                    obp = op.tile([128, cn], dt.float32, tag=f"obc{p}_{s}",
                                  name=f"obc{p}")
                    nc.vector.tensor_add(
                        obp[:], ap[:, cb:cb + cn],
                        biasf[:, ob + cb:ob + cb + cn])
                    nc.sync.dma_start(
                        o_d[t * 128:(t + 1) * 128, ob + cb:ob + cb + cn],
                        obp[:])

        for t in (0, 1):
            copy_out(t, [(accs[t][p][:], ob, on)
                         for p, (ob, on) in enumerate(O_SPLITS)])

        # t=2: finish its third o-bank, then copy out
        a2t2 = pacc.tile([128, 352], dt.float32, tag="a2", name="a2")
        for k in range(NK):
            nc.tensor.matmul(a2t2[:], xs_map[2][:, k],
                             wk_flat[k][:, 1024:1376],
                             start=(k == 0), stop=(k == NK - 1))
        copy_out(2, [(x01[:, 0:512], 0, 512),
                     (x01[:, 512:1024], 512, 512),
                     (a2t2[:], 1024, 352)])

        # ---- phase 2: remaining t-tiles, dense matmul stream ----
        for t in range(3, NT):
            xs = xp.tile([128, NK, 128], dt.float16, tag="xs", name="xs")
            nc.sync.dma_start(xs[:], xt_d[t])
            acc = [pacc.tile([128, on], dt.float32, tag=f"a{p}",
                             name=f"a{p}")
                   for p, (ob, on) in enumerate(O_SPLITS)]
            for k in range(NK):
                wk = wk_flat[k]
                for p, (ob, on) in enumerate(O_SPLITS):
                    nc.tensor.matmul(
                        acc[p][:], xs[:, k], wk[:, ob:ob + on],
                        start=(k == 0), stop=(k == NK - 1))
            copy_out(t, [(acc[p][:], ob, on)
                         for p, (ob, on) in enumerate(O_SPLITS)],
                     chunked=(t == NT - 1))

    nc.compile()
    return nc


def get_nc():
    if "nc" not in _CACHE:
        _CACHE["nc"] = _build()
    return _CACHE["nc"]


def make_in_maps(x, W_q, scale, zero, bias):
    x = np.ascontiguousarray(x, dtype=np.float32)
    W_q = np.ascontiguousarray(W_q, dtype=np.int32)
    bias = np.ascontiguousarray(bias, dtype=np.float32)

    # x.T tiled: [tt, ii, k, tj] fp16
    xt = np.ascontiguousarray(
        x.T.reshape(NK, 128, NT, 128).transpose(2, 1, 0, 3)
    ).astype(np.float16)
    # scale/zero: [172, 4096] -> [c, ii, kk, j] fp16 (KCH k-tiles per chunk)
    NCH = NK // KCH
    s2 = (scale.reshape(J, IN_F).T.reshape(NCH, KCH, 128, J)
          .transpose(0, 2, 1, 3).astype(np.float16))
    z2 = (zero.reshape(J, IN_F).T.reshape(NCH, KCH, 128, J)
          .transpose(0, 2, 1, 3).astype(np.float16))
    s2 = np.ascontiguousarray(s2)
    z2 = np.ascontiguousarray(z2)

    in_maps = []
    for c in range(NCORES):
        r0 = RPC * (c % 4)
        q = W_q[r0:r0 + RPC]                       # [8, 704512]
        nib = ((q >> 4) if c < 4 else q) & 0xF     # hi rows for cores 0-3
        nib = nib.astype(np.uint8).reshape(RPC, J, IN_F)
        nib = (nib.transpose(2, 0, 1).reshape(NCH, KCH, 128, RPC, J)
               .transpose(0, 2, 1, 3, 4))
        nib = np.ascontiguousarray(nib)
        in_maps.append({
            "nib": nib,
            "sc": s2,
            "z": z2,
            "xt": xt,
            "bias": bias[c * O_C:(c + 1) * O_C].reshape(1, O_C),
        })
    return in_maps


def kernel(x, W_q, scale, zero, bias):
    nc = get_nc()
    in_maps = make_in_maps(x, W_q, scale, zero, bias)
    res = run_bass_kernel_spmd(nc, in_maps, list(range(NCORES)))
    return np.concatenate(
        [res.results[c]["out"] for c in range(NCORES)], axis=1)


# revision 21
# speedup vs baseline: 2.0341x; 1.0053x over previous
"""HQQ 4-bit quantized linear on 8 Trainium2 NeuronCores (Bass/Tile).

out[4096, 11008] = x[4096, 4096] @ dequant(W_q, scale, zero).T + bias

Column-parallel: core c owns g_rows [8c, 8c+8) of the 64-row nibble
matrix, i.e. the contiguous output slice o in [1376c, 1376c+1376).
Within a core, output col = r*172 + j (r = local g_row, j in [0,172)),
input i = k*128 + ii; group g = j*4096 + i.

Host staging (layout/bit-extract only - all arithmetic stays on device):
  - nibble of interest extracted to uint8 and pre-transposed to
    [k, ii, r, j] so dequant runs directly in the matmul layout
  - x pre-transposed/tiled to [tt, ii, k, tj] fp16
  - scale/zero transposed to [k, ii, j] fp16

Device per core:
  phase 1: per k-tile, ACT casts nib u8->fp16, then DVE computes
           W = (nib - zero_bcast) * scale_bcast in two all-fp16 passes
           (2x DVE mode, broadcast APs along r) into resident fp16
           WT[128, 32, 8, 172].  Interleaved with the dequant, the PE
           accumulates t-tiles 0,1 (all 3 o-banks) and t-tile 2's first
           two o-banks -- all 8 PSUM banks -- so the PE stays ~busy
           through phase 1.
  phase 2: per t-tile, one 1MB DMA of x.T tiles, then 32k x 3 o-bank
           back-to-back fp16 matmuls accumulating in PSUM; bias added
           during the PSUM->SBUF copy on DVE (bias replicated across
           partitions once by a broadcast DMA).
"""

import numpy as np
from contextlib import ExitStack

import concourse.bacc as bacc
import concourse.bass as bass
import concourse.mybir as mybir
import concourse.tile as tile
from concourse.bass_utils import run_bass_kernel_spmd

dt = mybir.dt

TOKENS, IN_F, OUT_F, GS = 4096, 4096, 11008, 64
J = 172                               # groups per (g_row, i) plane
NCORES = 8
RPC = GS // NCORES                    # 8 g_rows per core
O_C = RPC * J                         # 1376 output cols per core
NT = TOKENS // 128                    # 32 token tiles
NK = IN_F // 128                      # 32 contraction tiles
O_SPLITS = ((0, 512), (512, 512), (1024, 352))   # psum o-tiles (1 bank each)
KCH = 2                               # k-tiles dequantized per DVE pass

_CACHE = {}


def _build():
    nc = bacc.Bacc("TRN2", target_bir_lowering=False, debug=False,
                   num_devices=NCORES)

    NCH = NK // KCH
    nib_d = nc.dram_tensor("nib", [NCH, 128, KCH, RPC, J], dt.uint8,
                           kind="ExternalInput")
    sc_d = nc.dram_tensor("sc", [NCH, 128, KCH, J], dt.float16,
                          kind="ExternalInput")
    z_d = nc.dram_tensor("z", [NCH, 128, KCH, J], dt.float16,
                         kind="ExternalInput")
    xt_d = nc.dram_tensor("xt", [NT, 128, NK, 128], dt.float16,
                          kind="ExternalInput")
    b_d = nc.dram_tensor("bias", [1, O_C], dt.float32, kind="ExternalInput")
    o_d = nc.dram_tensor("out", [TOKENS, O_C], dt.float32,
                         kind="ExternalOutput")

    with ExitStack() as ctx:
        tc = ctx.enter_context(tile.TileContext(nc))
        const = ctx.enter_context(tc.tile_pool(name="const", bufs=1))
        ph1 = ctx.enter_context(tc.tile_pool(name="ph1", bufs=3))
        xp = ctx.enter_context(tc.tile_pool(name="xp", bufs=3))
        op = ctx.enter_context(tc.tile_pool(name="op", bufs=3))
        pacc = ctx.enter_context(
            tc.tile_pool(name="pacc", bufs=2, space=bass.MemorySpace.PSUM))
        pacc2 = ctx.enter_context(
            tc.tile_pool(name="pacc2", bufs=1, space=bass.MemorySpace.PSUM))

        WT = const.tile([128, NK, RPC, J], dt.float16)   # resident W.T

        def fetch_ch(c, split=False):
            nib = ph1.tile([128, KCH, RPC, J], dt.uint8, tag="nib",
                           name="nib")
            if split:
                # first k-tile in its own DMA so dequant starts sooner
                for kk in range(KCH):
                    nc.sync.dma_start(nib[:, kk], nib_d[c][:, kk])
            else:
                nc.sync.dma_start(nib[:], nib_d[c])
            sct = ph1.tile([128, KCH, J], dt.float16, tag="sc", name="sct")
            nc.sync.dma_start(sct[:], sc_d[c])
            zt = ph1.tile([128, KCH, J], dt.float16, tag="z", name="zt")
            nc.sync.dma_start(zt[:], z_d[c])
            return nib, sct, zt

        # first dequant chunks ahead of the big x.T slabs so the DVE can
        # start immediately
        pre = {0: fetch_ch(0, split=True), 1: fetch_ch(1)}

        xs_map = {}
        for t in range(3):
            xs = xp.tile([128, NK, 128], dt.float16, tag="xs", name=f"xs{t}")
            nc.sync.dma_start(xs[:], xt_d[t])
            xs_map[t] = xs

        biasf = const.tile([128, O_C], dt.float32)
        nc.sync.dma_start(biasf[:], b_d[:].to_broadcast((128, O_C)))

        accs = {}
        for t in (0, 1):
            accs[t] = [pacc.tile([128, on], dt.float32, tag=f"a{p}",
                                 name=f"a{p}")
                       for p, (ob, on) in enumerate(O_SPLITS)]
        x01 = pacc2.tile([128, 1024], dt.float32, name="x01")

        # ---- phase 1: dequantize into WT + early matmuls (8 psum banks)
        wk_flat = {}
        NCH = NK // KCH

        def early_mms(k):
            wk = WT[:, k].opt()            # flat [128, O_C] view
            wk_flat[k] = wk
            se = dict(start=(k == 0), stop=(k == NK - 1))
            for t in (0, 1):
                for p, (ob, on) in enumerate(O_SPLITS):
                    nc.tensor.matmul(accs[t][p][:], xs_map[t][:, k],
                                     wk[:, ob:ob + on], **se)
            nc.tensor.matmul(x01[:, 0:512], xs_map[2][:, k],
                             wk[:, 0:512], **se)
            nc.tensor.matmul(x01[:, 512:1024], xs_map[2][:, k],
                             wk[:, 512:1024], **se)

        # chunk 0 fast path: per-k dequant straight from u8 (no ACT cast
        # on the critical chain; DVE reads u8 at 1x but latency wins)
        nib0, sct0, zt0 = pre.pop(0)
        for kk in range(KCH):
            d1 = ph1.tile([128, RPC, J], dt.float16, tag="d1", name="d1")
            nc.vector.tensor_sub(
                d1[:], nib0[:, kk],
                zt0[:, kk].unsqueeze(1).broadcast_to((128, RPC, J)))
            nc.vector.tensor_mul(
                WT[:, kk], d1[:],
                sct0[:, kk].unsqueeze(1).broadcast_to((128, RPC, J)))
            early_mms(kk)

        for c in range(1, NCH):
            nib, sct, zt = pre.pop(c) if c in pre else fetch_ch(c)
            nibf = ph1.tile([128, KCH, RPC, J], dt.float16, tag="nibf",
                            name="nibf")
            nc.scalar.copy(nibf[:], nib[:])
            d = ph1.tile([128, KCH, RPC, J], dt.float16, tag="d", name="d")
            nc.vector.tensor_sub(
                d[:], nibf[:],
                zt[:].unsqueeze(2).broadcast_to((128, KCH, RPC, J)))
            nc.vector.tensor_mul(
                WT[:, c * KCH:(c + 1) * KCH], d[:],
                sct[:].unsqueeze(2).broadcast_to((128, KCH, RPC, J)))
            for k in range(c * KCH, (c + 1) * KCH):
                early_mms(k)

        def copy_out(t, psums, chunked=False):
            for p, (ap, ob, on) in enumerate(psums):
                if not chunked:
                    obp = op.tile([128, on], dt.float32, tag=f"ob{p}",
                                  name=f"ob{p}")
                    nc.vector.tensor_add(obp[:], ap, biasf[:, ob:ob + on])
                    nc.sync.dma_start(
                        o_d[t * 128:(t + 1) * 128, ob:ob + on], obp[:])
                    continue
                h = on // 2
                for s, (cb, cn) in enumerate(((0, h), (h, on - h))):
                    obp = op.tile([128, cn], dt.float32, tag=f"obc{p}_{s}",
                                  name=f"obc{p}")
                    nc.vector.tensor_add(
                        obp[:], ap[:, cb:cb + cn],
                        biasf[:, ob + cb:ob + cb + cn])
                    nc.sync.dma_start(
                        o_d[t * 128:(t + 1) * 128, ob + cb:ob + cb + cn],
                        obp[:])

        for t in (0, 1):
            copy_out(t, [(accs[t][p][:], ob, on)
                         for p, (ob, on) in enumerate(O_SPLITS)])

        # t=2: finish its third o-bank, then copy out
        a2t2 = pacc.tile([128, 352], dt.float32, tag="a2", name="a2")
        for k in range(NK):
            nc.tensor.matmul(a2t2[:], xs_map[2][:, k],
                             wk_flat[k][:, 1024:1376],
                             start=(k == 0), stop=(k == NK - 1))
        copy_out(2, [(x01[:, 0:512], 0, 512),
                     (x01[:, 512:1024], 512, 512),
                     (a2t2[:], 1024, 352)])

        # ---- phase 2: remaining t-tiles, dense matmul stream ----
        for t in range(3, NT):
            xs = xp.tile([128, NK, 128], dt.float16, tag="xs", name="xs")
            nc.sync.dma_start(xs[:], xt_d[t])
            acc = [pacc.tile([128, on], dt.float32, tag=f"a{p}",
                             name=f"a{p}")
                   for p, (ob, on) in enumerate(O_SPLITS)]
            for k in range(NK):
                wk = wk_flat[k]
                for p, (ob, on) in enumerate(O_SPLITS):
                    nc.tensor.matmul(
                        acc[p][:], xs[:, k], wk[:, ob:ob + on],
                        start=(k == 0), stop=(k == NK - 1))
            copy_out(t, [(acc[p][:], ob, on)
                         for p, (ob, on) in enumerate(O_SPLITS)],
                     chunked=(t == NT - 1))

    nc.compile()
    return nc


def get_nc():
    if "nc" not in _CACHE:
        _CACHE["nc"] = _build()
    return _CACHE["nc"]


def make_in_maps(x, W_q, scale, zero, bias):
    x = np.ascontiguousarray(x, dtype=np.float32)
    W_q = np.ascontiguousarray(W_q, dtype=np.int32)
    bias = np.ascontiguousarray(bias, dtype=np.float32)

    # x.T tiled: [tt, ii, k, tj] fp16
    xt = np.ascontiguousarray(
        x.T.reshape(NK, 128, NT, 128).transpose(2, 1, 0, 3)
    ).astype(np.float16)
    # scale/zero: [172, 4096] -> [c, ii, kk, j] fp16 (KCH k-tiles per chunk)
    NCH = NK // KCH
    s2 = (scale.reshape(J, IN_F).T.reshape(NCH, KCH, 128, J)
          .transpose(0, 2, 1, 3).astype(np.float16))
    z2 = (zero.reshape(J, IN_F).T.reshape(NCH, KCH, 128, J)
          .transpose(0, 2, 1, 3).astype(np.float16))
    s2 = np.ascontiguousarray(s2)
    z2 = np.ascontiguousarray(z2)

    in_maps = []
    for c in range(NCORES):
        r0 = RPC * (c % 4)
        q = W_q[r0:r0 + RPC]                       # [8, 704512]
        nib = ((q >> 4) if c < 4 else q) & 0xF     # hi rows for cores 0-3
        nib = nib.astype(np.uint8).reshape(RPC, J, IN_F)
        nib = (nib.transpose(2, 0, 1).reshape(NCH, KCH, 128, RPC, J)
               .transpose(0, 2, 1, 3, 4))
        nib = np.ascontiguousarray(nib)
        in_maps.append({
            "nib": nib,
            "sc": s2,
            "z": z2,
            "xt": xt,
            "bias": bias[c * O_C:(c + 1) * O_C].reshape(1, O_C),
        })
    return in_maps


def kernel(x, W_q, scale, zero, bias):
    nc = get_nc()
    in_maps = make_in_maps(x, W_q, scale, zero, bias)
    res = run_bass_kernel_spmd(nc, in_maps, list(range(NCORES)))
    return np.concatenate(
        [res.results[c]["out"] for c in range(NCORES)], axis=1)
